# revision 5
# baseline (speedup 1.0000x reference)
"""ConvKAN fused kernel for Trainium2, 8-core data-parallel over batch.

Reformulation: the reference's B-spline basis (Cox-de Boor with stale
lower-degree entries) is a fixed linear map S of 17 truncated-power
features of u = sigmoid(x):
    f3_p=relu(u-p/11)^3 p=0..10, f2_p=relu(u-p/11)^2 p=8..10,
    f1_p=relu(u-p/11) p=9,10, f0=(u>=10/11)
so  spline + conv  ==  one 3x3 conv over 18 per-channel feature maps
(17 basis features + raw x) with host-folded weights cp2 = [cp @ S | conv_w].
BatchNorm statistics are all-reduced across the 8 cores.

Matmuls run in full-fp32 PE mode: the folded weights have large canceling
coefficients (c3 = 11^3/6 scale), which float32r's ~11-bit operand rounding
amplifies ~100x; fp32 mode is exact (measured 2e-7).

conv_b is ignored: BatchNorm(x + const) == BatchNorm(x).
"""
import numpy as np

import concourse.bass as bass
import concourse.tile as tile
import concourse.mybir as mybir
from concourse import bacc
from concourse.bass_utils import run_bass_kernel_spmd

# ---- problem constants (hardcoded per contract) ----
B, C, O, HH, WW = 8, 64, 128, 56, 56
KK = 3
M = 11
EPS = 1e-5
N_CORES = 8
PW = WW + 2            # 58 padded width
PCOLS = PW * PW        # 3364 padded spatial
L = HH * WW            # 3136 outputs per channel
N_TILES = 9            # contraction tiles per tap (18 features x 64 ch / 128)
GROUPS = 7             # output row groups of 8 rows
GW = 8 * PW            # 464: col stride between groups
CHW = 580              # feature chunk width = 10 padded rows
NMM_FREE = 462         # matmul moving free dim per group (58*8-2)
PSUM_W = 464

# 3-set bf16 split matmuls: W=W_hi+W_lo, F=F_hi+F_lo (bf16 pairs; products are
# exact in f32 PSUM; only the ~2^-17-scale W_lo*F_lo term is dropped).
# 243 bf16 MMs/group (~197ns) vs 81 f32 MMs (~1152ns): ~2x faster PE.
SPLIT_BF16 = True

_KNOTS32 = np.linspace(0.0, 1.0, M + 1).astype(np.float32)

_cache = {}


def _build_S():
    """S: [11 basis, 17 features] float64 (exact rational knots)."""
    h = 1.0 / M
    S = np.zeros((M, 17), dtype=np.float64)
    c3 = 1.0 / (6.0 * h ** 3)
    b4 = [1.0, -4.0, 6.0, -4.0, 1.0]
    for j in range(8):
        for r in range(5):
            if j + r <= 10:
                S[j, j + r] += c3 * b4[r]
    c2 = 1.0 / (2.0 * h ** 2)
    b3 = [1.0, -3.0, 3.0, -1.0]
    for r in range(4):
        if 8 + r <= 10:
            S[8, 11 + r] += c2 * b3[r]
    S[9, 14] += M
    S[9, 15] += -2.0 * M
    S[10, 16] = 1.0
    return S


def _build_weights(control_points, conv_w):
    """-> wts [9 taps][9 tiles][128 rows, 128 o] f32.

    Contraction row map (tile t): rows 0..63 = feature 2t, ch c=row;
    rows 64..127 = feature 2t+1, ch c=row-64. Feature order matches the
    on-chip tiles: 0..10 cubes, 11..13 squares(p=8,9,10), 14..15 relu(p=9,10),
    16 step, 17 raw x.
    """
    S = _build_S()
    cp2 = np.zeros((O, C, KK * KK, 18), dtype=np.float64)
    cp2[..., :17] = control_points.astype(np.float64) @ S
    cp2[..., 17] = conv_w.reshape(O, C, KK * KK).astype(np.float64)
    wts = np.zeros((KK * KK, N_TILES, 128, 128), dtype=np.float32)
    for k in range(KK * KK):
        for t in range(N_TILES):
            wts[k, t, 0:64, :] = cp2[:, :, k, 2 * t].T        # [c, o]
            wts[k, t, 64:128, :] = cp2[:, :, k, 2 * t + 1].T
    return wts


def _build_nc():
    nc = bacc.Bacc("TRN2", target_bir_lowering=False, debug=False,
                   num_devices=N_CORES)
    dt = mybir.dt.float32
    bt16 = mybir.dt.bfloat16
    xpad_d = nc.dram_tensor("xpad", [C, PCOLS], dt, kind="ExternalInput").ap()
    if SPLIT_BF16:
        wts_d = nc.dram_tensor("wts", [2 * KK * KK * N_TILES * 128, 128], bt16,
                               kind="ExternalInput").ap()
    else:
        wts_d = nc.dram_tensor("wts", [KK * KK * N_TILES * 128, 128], dt,
                               kind="ExternalInput").ap()
    gam_d = nc.dram_tensor("gam", [O, 1], dt, kind="ExternalInput").ap()
    bet_d = nc.dram_tensor("bet", [O, 1], dt, kind="ExternalInput").ap()
    out_d = nc.dram_tensor("out", [O, L], dt, kind="ExternalOutput").ap()

    t32 = _KNOTS32

    with tile.TileContext(nc) as tc:
        with (
            tc.tile_pool(name="wpool", bufs=1) as wpool,
            tc.tile_pool(name="fpool", bufs=2) as fpool,
            tc.tile_pool(name="spool", bufs=2) as spool,
            tc.tile_pool(name="cpool", bufs=1) as cpool,
            tc.tile_pool(name="psum", bufs=2, space="PSUM") as pp,
            tc.tile_pool(name="dram", bufs=1, space="DRAM") as dram,
        ):
            # ---- persistent: weights, output, stats, biases ----
            wdt = bt16 if SPLIT_BF16 else dt
            nsets_w = 2 if SPLIT_BF16 else 1
            w_sb = [[[wpool.tile([128, 128], wdt, tag=f"w{s}_{k}_{t}",
                                 name=f"w{s}_{k}_{t}")
                      for t in range(N_TILES)] for k in range(KK * KK)]
                    for s in range(nsets_w)]
            for s in range(nsets_w):
                for k in range(KK * KK):
                    for t in range(N_TILES):
                        r0 = ((s * KK * KK + k) * N_TILES + t) * 128
                        nc.sync.dma_start(w_sb[s][k][t][:], wts_d[r0:r0 + 128, :])
            out_sb = cpool.tile([128, L], dt, tag="out_sb")
            sums = cpool.tile([128, GROUPS], dt, tag="sums")
            sqs = cpool.tile([128, GROUPS], dt, tag="sqs")
            gam_sb = cpool.tile([128, 1], dt, tag="gam")
            bet_sb = cpool.tile([128, 1], dt, tag="bet")
            nc.sync.dma_start(gam_sb[:], gam_d[:])
            nc.sync.dma_start(bet_sb[:], bet_d[:])

            # per-pair bias tiles: -(t_a) rows 0..63, -(t_b) rows 64..127
            pair_ts = [(0, 1), (2, 3), (4, 5), (6, 7), (8, 9),   # cubes T0-4
                       (10, 8),                                  # T5 cube10|sq8
                       (9, 10),                                  # T6 sq9|sq10
                       (9, 10)]                                  # T7 lin9|lin10
            biases = []
            for i, (pa, pb) in enumerate(pair_ts):
                bt = cpool.tile([128, 1], dt, tag=f"bias{i}")
                nc.gpsimd.memset(bt[0:64, :], -float(t32[pa]))
                nc.gpsimd.memset(bt[64:128, :], -float(t32[pb]))
                biases.append(bt)

            AF = mybir.ActivationFunctionType
            ALU = mybir.AluOpType

            # ---- main loop: 7 chunks/groups ----
            for g in range(GROUPS):
                c0 = g * GW          # chunk covers xpad cols [c0, c0+580)
                x2 = spool.tile([128, CHW], dt, tag="x2")
                nc.sync.dma_start(x2[0:64, :], xpad_d[:, c0:c0 + CHW])
                nc.sync.dma_start(x2[64:128, :], xpad_d[:, c0:c0 + CHW])
                u2 = spool.tile([128, CHW], dt, tag="u2")
                nc.scalar.activation(u2[:], x2[:], AF.Sigmoid)

                F = [fpool.tile([128, CHW], dt, tag=f"F{t}", name=f"F{t}")
                     for t in range(N_TILES)]
                # cubes T0..T4: F = relu(u+b) * (u+b)^2
                for i in range(5):
                    v = spool.tile([128, CHW], dt, tag="v")
                    q = spool.tile([128, CHW], dt, tag="q")
                    nc.scalar.activation(v[:], u2[:], AF.Relu, bias=biases[i][:])
                    nc.scalar.activation(q[:], u2[:], AF.Square, bias=biases[i][:])
                    nc.vector.tensor_mul(F[i][:], v[:], q[:])
                # T5: top f3_10 = v^3, bottom f2_8 = v^2
                v5 = spool.tile([128, CHW], dt, tag="v")
                nc.scalar.activation(v5[:], u2[:], AF.Relu, bias=biases[5][:])
                nc.scalar.activation(F[5][:], v5[:], AF.Square)
                nc.vector.tensor_mul(F[5][0:64, :], F[5][0:64, :], v5[0:64, :])
                # T6: squares of relu (p=9,10)
                v6 = spool.tile([128, CHW], dt, tag="v")
                nc.scalar.activation(v6[:], u2[:], AF.Relu, bias=biases[6][:])
                nc.scalar.activation(F[6][:], v6[:], AF.Square)
                # T7: linear relu (p=9,10)
                nc.scalar.activation(F[7][:], u2[:], AF.Relu, bias=biases[7][:])
                # T8: top step(u>=t10), bottom raw x
                nc.sync.dma_start(F[8][64:128, :], xpad_d[:, c0:c0 + CHW])
                nc.vector.tensor_scalar(F[8][0:64, :], u2[0:64, :],
                                        float(t32[10]), None, ALU.is_ge)

                if SPLIT_BF16:
                    Fh = [fpool.tile([128, CHW], bt16, tag=f"Fh{t}", name=f"Fh{t}")
                          for t in range(N_TILES)]
                    Fl = [fpool.tile([128, CHW], bt16, tag=f"Fl{t}", name=f"Fl{t}")
                          for t in range(N_TILES)]
                    for t in range(N_TILES):
                        nc.vector.tensor_copy(Fh[t][:], F[t][:])
                        fl32 = spool.tile([128, CHW], dt, tag="fl32", name="fl32")
                        nc.vector.tensor_sub(fl32[:], F[t][:], Fh[t][:])
                        nc.vector.tensor_copy(Fl[t][:], fl32[:])
                    mm_sets = [(0, Fh), (0, Fl), (1, Fh)]  # skip lo*lo
                else:
                    mm_sets = [(0, F)]

                # ---- accumulating matmuls for this 8-row output group ----
                ps = pp.tile([128, PSUM_W], dt, tag="ps")
                nmm = len(mm_sets) * KK * KK * N_TILES
                i_mm = 0
                for ws, Fset in mm_sets:
                    for dh in range(KK):
                        for dw in range(KK):
                            k = dh * KK + dw
                            off = dh * PW + dw
                            for t in range(N_TILES):
                                nc.tensor.matmul(
                                    ps[:, 0:NMM_FREE], w_sb[ws][k][t][:],
                                    Fset[t][:, off:off + NMM_FREE],
                                    start=(i_mm == 0), stop=(i_mm == nmm - 1))
                                i_mm += 1

                # ---- extract valid cols + BN partial stats ----
                psv = ps[:].rearrange("p (r w) -> p r w", w=PW)[:, :, 0:WW]
                ov = out_sb[:, g * 8 * WW:(g + 1) * 8 * WW].rearrange(
                    "p (r w) -> p r w", w=WW)
                nc.scalar.activation(ov, psv, AF.Copy,
                                     accum_out=sums[:, g:g + 1])
                sqt = spool.tile([128, 8 * WW], dt, tag="sqt")
                sqv = sqt[:].rearrange("p (r w) -> p r w", w=WW)
                nc.scalar.activation(sqv, psv, AF.Square,
                                     accum_out=sqs[:, g:g + 1])

            # ---- BN: reduce partials, all-reduce, normalize ----
            stats = cpool.tile([128, 2], dt, tag="stats")
            nc.vector.reduce_sum(stats[:, 0:1], sums[:], axis=mybir.AxisListType.X)
            nc.vector.reduce_sum(stats[:, 1:2], sqs[:], axis=mybir.AxisListType.X)
            cc_in = dram.tile([128, 2], dt)
            cc_out = dram.tile([128, 2], dt)
            nc.sync.dma_start(cc_in[:], stats[:])
            nc.gpsimd.collective_compute(
                "AllReduce", ALU.add, replica_groups=[list(range(N_CORES))],
                ins=[cc_in.opt()], outs=[cc_out.opt()])
            gst = cpool.tile([128, 2], dt, tag="gst")
            nc.sync.dma_start(gst[:], cc_out[:])

            inv_n = 1.0 / float(B * L)
            mean = cpool.tile([128, 1], dt, tag="mean")
            veps = cpool.tile([128, 1], dt, tag="veps")
            t1 = cpool.tile([128, 1], dt, tag="t1")
            nc.vector.tensor_scalar(mean[:], gst[:, 0:1], inv_n, None, ALU.mult)
            nc.vector.tensor_scalar(veps[:], gst[:, 1:2], inv_n, None, ALU.mult)
            nc.vector.tensor_mul(t1[:], mean[:], mean[:])
            nc.vector.tensor_sub(veps[:], veps[:], t1[:])
            nc.vector.tensor_scalar(veps[:], veps[:], EPS, None, ALU.add)
            y = cpool.tile([128, 1], dt, tag="y")
            nc.vector.reciprocal(y[:], veps[:])
            nc.scalar.activation(y[:], y[:], AF.Sqrt)
            # one Newton step: y *= 1.5 - 0.5*veps*y^2  (guards Rsqrt table error)
            nc.vector.tensor_mul(t1[:], y[:], y[:])
            nc.vector.tensor_mul(t1[:], t1[:], veps[:])
            nc.vector.tensor_scalar(t1[:], t1[:], -0.5, 1.5, ALU.mult, ALU.add)
            nc.vector.tensor_mul(y[:], y[:], t1[:])
            scale = cpool.tile([128, 1], dt, tag="scale")
            shift = cpool.tile([128, 1], dt, tag="shift")
            nc.vector.tensor_mul(scale[:], y[:], gam_sb[:])
            nc.vector.tensor_mul(t1[:], mean[:], scale[:])
            nc.vector.tensor_sub(shift[:], bet_sb[:], t1[:])
            nc.vector.tensor_scalar(out_sb[:], out_sb[:], scale[:, 0:1],
                                    shift[:, 0:1], ALU.mult, ALU.add)
            nc.sync.dma_start(out_d[:], out_sb[:])
    nc.compile()
    return nc


def kernel(**inputs):
    x = np.ascontiguousarray(np.asarray(inputs["x"], dtype=np.float32))
    cp = np.asarray(inputs["control_points"], dtype=np.float32)
    conv_w = np.asarray(inputs["conv_w"], dtype=np.float32)
    gam = np.asarray(inputs["bn_gamma"], dtype=np.float32)
    bet = np.asarray(inputs["bn_beta"], dtype=np.float32)

    wts_f32 = _build_weights(cp, conv_w).reshape(KK * KK * N_TILES * 128, 128)
    if SPLIT_BF16:
        import ml_dtypes
        w_hi = wts_f32.astype(ml_dtypes.bfloat16)
        w_lo = (wts_f32 - w_hi.astype(np.float32)).astype(ml_dtypes.bfloat16)
        wts = np.ascontiguousarray(np.concatenate([w_hi, w_lo], axis=0))
    else:
        wts = np.ascontiguousarray(wts_f32)
    xpad = np.zeros((B, C, PW, PW), dtype=np.float32)
    xpad[:, :, 1:-1, 1:-1] = x
    xpad = xpad.reshape(B, C, PCOLS)

    if "nc" not in _cache:
        _cache["nc"] = _build_nc()
    nc = _cache["nc"]

    in_maps = [{"xpad": xpad[b], "wts": wts, "gam": gam.reshape(O, 1),
                "bet": bet.reshape(O, 1)} for b in range(B)]
    res = run_bass_kernel_spmd(nc, in_maps, list(range(N_CORES)))
    out = np.stack([res.results[b]["out"].reshape(O, HH, WW)
                    for b in range(B)], axis=0)
    return out.astype(np.float32)


# revision 6
# speedup vs baseline: 1.5634x; 1.5634x over previous
"""ConvKAN fused kernel for Trainium2, 8-core data-parallel over batch.

Reformulation: the reference's B-spline basis (Cox-de Boor with stale
lower-degree entries) is a fixed linear map S of 17 truncated-power
features of u = sigmoid(x):
    f3_p=relu(u-p/11)^3 p=0..10, f2_p=relu(u-p/11)^2 p=8..10,
    f1_p=relu(u-p/11) p=9,10, f0=(u>=10/11)
so  spline + conv  ==  one 3x3 conv over 18 per-channel feature maps
(17 basis features + raw x) with host-folded weights cp2 = [cp @ S | conv_w].
BatchNorm statistics are all-reduced across the 8 cores.

The folded weights have large canceling coefficients (c3 = 11^3/6 scale), so
reduced-precision operands are amplified ~100x (bf16 -> ~1.8 absmax error,
float32r's ~11-bit operand rounding -> ~0.1). Matmuls therefore run as 3-set
bf16 split products (W_hi/W_lo x F_hi/F_lo, lo*lo dropped): every bf16*bf16
product is exact in the f32 PSUM, leaving only ~2^-17-scale residuals
(measured 6.2e-4 relative absmax vs the f32 reference).

conv_b is ignored: BatchNorm(x + const) == BatchNorm(x).
"""
import numpy as np

import concourse.bass as bass
import concourse.tile as tile
import concourse.mybir as mybir
from concourse import bacc
from concourse.bass_utils import run_bass_kernel_spmd

# ---- problem constants (hardcoded per contract) ----
B, C, O, HH, WW = 8, 64, 128, 56, 56
KK = 3
M = 11
EPS = 1e-5
N_CORES = 8
PW = WW + 2            # 58 padded width
PCOLS = PW * PW        # 3364 padded spatial
L = HH * WW            # 3136 outputs per channel
N_TILES = 9            # contraction tiles per tap (18 features x 64 ch / 128)
GROUPS = 7             # output row groups of 8 rows
GW = 8 * PW            # 464: col stride between groups
CHW = 580              # feature chunk width = 10 padded rows
NMM_FREE = 462         # matmul moving free dim per group (58*8-2)
PSUM_W = 464

# 3-set bf16 split matmuls: W=W_hi+W_lo, F=F_hi+F_lo (bf16 pairs; products are
# exact in f32 PSUM; only the ~2^-17-scale W_lo*F_lo term is dropped).
# 243 bf16 MMs/group (~197ns) vs 81 f32 MMs (~1152ns): ~2x faster PE.
SPLIT_BF16 = True

_KNOTS32 = np.linspace(0.0, 1.0, M + 1).astype(np.float32)

_cache = {}


def _build_S():
    """S: [11 basis, 17 features] float64 (exact rational knots)."""
    h = 1.0 / M
    S = np.zeros((M, 17), dtype=np.float64)
    c3 = 1.0 / (6.0 * h ** 3)
    b4 = [1.0, -4.0, 6.0, -4.0, 1.0]
    for j in range(8):
        for r in range(5):
            if j + r <= 10:
                S[j, j + r] += c3 * b4[r]
    c2 = 1.0 / (2.0 * h ** 2)
    b3 = [1.0, -3.0, 3.0, -1.0]
    for r in range(4):
        if 8 + r <= 10:
            S[8, 11 + r] += c2 * b3[r]
    S[9, 14] += M
    S[9, 15] += -2.0 * M
    S[10, 16] = 1.0
    return S


def _build_weights(control_points, conv_w):
    """-> wts [9 taps][9 tiles][128 rows, 128 o] f32.

    Contraction row map (tile t): rows 0..63 = feature 2t, ch c=row;
    rows 64..127 = feature 2t+1, ch c=row-64. Feature order matches the
    on-chip tiles: 0..10 cubes, 11..13 squares(p=8,9,10), 14..15 relu(p=9,10),
    16 step, 17 raw x.
    """
    S = _build_S()
    cp2 = np.zeros((O, C, KK * KK, 18), dtype=np.float64)
    cp2[..., :17] = control_points.astype(np.float64) @ S
    cp2[..., 17] = conv_w.reshape(O, C, KK * KK).astype(np.float64)
    wts = np.zeros((KK * KK, N_TILES, 128, 128), dtype=np.float32)
    for k in range(KK * KK):
        for t in range(N_TILES):
            wts[k, t, 0:64, :] = cp2[:, :, k, 2 * t].T        # [c, o]
            wts[k, t, 64:128, :] = cp2[:, :, k, 2 * t + 1].T
    return wts


def _build_nc():
    nc = bacc.Bacc("TRN2", target_bir_lowering=False, debug=False,
                   num_devices=N_CORES)
    dt = mybir.dt.float32
    bt16 = mybir.dt.bfloat16
    xpad_d = nc.dram_tensor("xpad", [C, PCOLS], dt, kind="ExternalInput").ap()
    if SPLIT_BF16:
        wts_d = nc.dram_tensor("wts", [2 * KK * KK * N_TILES * 128, 128], bt16,
                               kind="ExternalInput").ap()
    else:
        wts_d = nc.dram_tensor("wts", [KK * KK * N_TILES * 128, 128], dt,
                               kind="ExternalInput").ap()
    gam_d = nc.dram_tensor("gam", [O, 1], dt, kind="ExternalInput").ap()
    bet_d = nc.dram_tensor("bet", [O, 1], dt, kind="ExternalInput").ap()
    out_d = nc.dram_tensor("out", [O, L], dt, kind="ExternalOutput").ap()

    t32 = _KNOTS32

    with tile.TileContext(nc) as tc:
        with (
            tc.tile_pool(name="wpool", bufs=1) as wpool,
            tc.tile_pool(name="fpool", bufs=2) as fpool,
            tc.tile_pool(name="spool", bufs=2) as spool,
            tc.tile_pool(name="cpool", bufs=1) as cpool,
            tc.tile_pool(name="psum", bufs=2, space="PSUM") as pp,
            tc.tile_pool(name="dram", bufs=1, space="DRAM") as dram,
        ):
            # ---- persistent: weights, output, stats, biases ----
            wdt = bt16 if SPLIT_BF16 else dt
            nsets_w = 2 if SPLIT_BF16 else 1
            w_sb = [[[wpool.tile([128, 128], wdt, tag=f"w{s}_{k}_{t}",
                                 name=f"w{s}_{k}_{t}")
                      for t in range(N_TILES)] for k in range(KK * KK)]
                    for s in range(nsets_w)]
            for s in range(nsets_w):
                for k in range(KK * KK):
                    for t in range(N_TILES):
                        r0 = ((s * KK * KK + k) * N_TILES + t) * 128
                        nc.sync.dma_start(w_sb[s][k][t][:], wts_d[r0:r0 + 128, :])
            out_sb = cpool.tile([128, L], dt, tag="out_sb")
            sums = cpool.tile([128, GROUPS], dt, tag="sums")
            sqs = cpool.tile([128, GROUPS], dt, tag="sqs")
            gam_sb = cpool.tile([128, 1], dt, tag="gam")
            bet_sb = cpool.tile([128, 1], dt, tag="bet")
            nc.sync.dma_start(gam_sb[:], gam_d[:])
            nc.sync.dma_start(bet_sb[:], bet_d[:])

            # per-pair bias tiles: -(t_a) rows 0..63, -(t_b) rows 64..127
            pair_ts = [(0, 1), (2, 3), (4, 5), (6, 7), (8, 9),   # cubes T0-4
                       (10, 8),                                  # T5 cube10|sq8
                       (9, 10),                                  # T6 sq9|sq10
                       (9, 10)]                                  # T7 lin9|lin10
            biases = []
            for i, (pa, pb) in enumerate(pair_ts):
                bt = cpool.tile([128, 1], dt, tag=f"bias{i}")
                nc.gpsimd.memset(bt[0:64, :], -float(t32[pa]))
                nc.gpsimd.memset(bt[64:128, :], -float(t32[pb]))
                biases.append(bt)

            AF = mybir.ActivationFunctionType
            ALU = mybir.AluOpType

            # ---- main loop: 7 chunks/groups ----
            for g in range(GROUPS):
                c0 = g * GW          # chunk covers xpad cols [c0, c0+580)
                x2 = spool.tile([128, CHW], dt, tag="x2")
                nc.sync.dma_start(x2[0:64, :], xpad_d[:, c0:c0 + CHW])
                nc.sync.dma_start(x2[64:128, :], xpad_d[:, c0:c0 + CHW])
                u2 = spool.tile([128, CHW], dt, tag="u2")
                nc.scalar.activation(u2[:], x2[:], AF.Sigmoid)

                F = [fpool.tile([128, CHW], dt, tag=f"F{t}", name=f"F{t}")
                     for t in range(N_TILES)]
                # cubes T0..T4: F = relu(u+b) * (u+b)^2
                for i in range(5):
                    v = spool.tile([128, CHW], dt, tag="v")
                    q = spool.tile([128, CHW], dt, tag="q")
                    nc.scalar.activation(v[:], u2[:], AF.Relu, bias=biases[i][:])
                    nc.scalar.activation(q[:], u2[:], AF.Square, bias=biases[i][:])
                    nc.vector.tensor_mul(F[i][:], v[:], q[:])
                # T5: top f3_10 = v^3, bottom f2_8 = v^2
                v5 = spool.tile([128, CHW], dt, tag="v")
                nc.scalar.activation(v5[:], u2[:], AF.Relu, bias=biases[5][:])
                nc.scalar.activation(F[5][:], v5[:], AF.Square)
                nc.vector.tensor_mul(F[5][0:64, :], F[5][0:64, :], v5[0:64, :])
                # T6: squares of relu (p=9,10)
                v6 = spool.tile([128, CHW], dt, tag="v")
                nc.scalar.activation(v6[:], u2[:], AF.Relu, bias=biases[6][:])
                nc.scalar.activation(F[6][:], v6[:], AF.Square)
                # T7: linear relu (p=9,10)
                nc.scalar.activation(F[7][:], u2[:], AF.Relu, bias=biases[7][:])
                # T8: top step(u>=t10), bottom raw x
                nc.sync.dma_start(F[8][64:128, :], xpad_d[:, c0:c0 + CHW])
                nc.vector.tensor_scalar(F[8][0:64, :], u2[0:64, :],
                                        float(t32[10]), None, ALU.is_ge)

                if SPLIT_BF16:
                    Fh = [fpool.tile([128, CHW], bt16, tag=f"Fh{t}", name=f"Fh{t}")
                          for t in range(N_TILES)]
                    Fl = [fpool.tile([128, CHW], bt16, tag=f"Fl{t}", name=f"Fl{t}")
                          for t in range(N_TILES)]
                    for t in range(N_TILES):
                        nc.vector.tensor_copy(Fh[t][:], F[t][:])
                        fl32 = spool.tile([128, CHW], dt, tag="fl32", name="fl32")
                        nc.vector.tensor_sub(fl32[:], F[t][:], Fh[t][:])
                        nc.vector.tensor_copy(Fl[t][:], fl32[:])
                    mm_sets = [(0, Fh), (0, Fl), (1, Fh)]  # skip lo*lo
                else:
                    mm_sets = [(0, F)]

                # ---- accumulating matmuls for this 8-row output group ----
                ps = pp.tile([128, PSUM_W], dt, tag="ps")
                nmm = len(mm_sets) * KK * KK * N_TILES
                i_mm = 0
                for ws, Fset in mm_sets:
                    for dh in range(KK):
                        for dw in range(KK):
                            k = dh * KK + dw
                            off = dh * PW + dw
                            for t in range(N_TILES):
                                nc.tensor.matmul(
                                    ps[:, 0:NMM_FREE], w_sb[ws][k][t][:],
                                    Fset[t][:, off:off + NMM_FREE],
                                    start=(i_mm == 0), stop=(i_mm == nmm - 1))
                                i_mm += 1

                # ---- extract valid cols + BN partial stats ----
                psv = ps[:].rearrange("p (r w) -> p r w", w=PW)[:, :, 0:WW]
                ov = out_sb[:, g * 8 * WW:(g + 1) * 8 * WW].rearrange(
                    "p (r w) -> p r w", w=WW)
                nc.scalar.activation(ov, psv, AF.Copy,
                                     accum_out=sums[:, g:g + 1])
                sqt = spool.tile([128, 8 * WW], dt, tag="sqt")
                sqv = sqt[:].rearrange("p (r w) -> p r w", w=WW)
                nc.scalar.activation(sqv, psv, AF.Square,
                                     accum_out=sqs[:, g:g + 1])

            # ---- BN: reduce partials, all-reduce, normalize ----
            stats = cpool.tile([128, 2], dt, tag="stats")
            nc.vector.reduce_sum(stats[:, 0:1], sums[:], axis=mybir.AxisListType.X)
            nc.vector.reduce_sum(stats[:, 1:2], sqs[:], axis=mybir.AxisListType.X)
            cc_in = dram.tile([128, 2], dt)
            cc_out = dram.tile([128, 2], dt)
            nc.sync.dma_start(cc_in[:], stats[:])
            nc.gpsimd.collective_compute(
                "AllReduce", ALU.add, replica_groups=[list(range(N_CORES))],
                ins=[cc_in.opt()], outs=[cc_out.opt()])
            gst = cpool.tile([128, 2], dt, tag="gst")
            nc.sync.dma_start(gst[:], cc_out[:])

            inv_n = 1.0 / float(B * L)
            mean = cpool.tile([128, 1], dt, tag="mean")
            veps = cpool.tile([128, 1], dt, tag="veps")
            t1 = cpool.tile([128, 1], dt, tag="t1")
            nc.vector.tensor_scalar(mean[:], gst[:, 0:1], inv_n, None, ALU.mult)
            nc.vector.tensor_scalar(veps[:], gst[:, 1:2], inv_n, None, ALU.mult)
            nc.vector.tensor_mul(t1[:], mean[:], mean[:])
            nc.vector.tensor_sub(veps[:], veps[:], t1[:])
            nc.vector.tensor_scalar(veps[:], veps[:], EPS, None, ALU.add)
            y = cpool.tile([128, 1], dt, tag="y")
            nc.vector.reciprocal(y[:], veps[:])
            nc.scalar.activation(y[:], y[:], AF.Sqrt)
            # one Newton step: y *= 1.5 - 0.5*veps*y^2  (guards Rsqrt table error)
            nc.vector.tensor_mul(t1[:], y[:], y[:])
            nc.vector.tensor_mul(t1[:], t1[:], veps[:])
            nc.vector.tensor_scalar(t1[:], t1[:], -0.5, 1.5, ALU.mult, ALU.add)
            nc.vector.tensor_mul(y[:], y[:], t1[:])
            scale = cpool.tile([128, 1], dt, tag="scale")
            shift = cpool.tile([128, 1], dt, tag="shift")
            nc.vector.tensor_mul(scale[:], y[:], gam_sb[:])
            nc.vector.tensor_mul(t1[:], mean[:], scale[:])
            nc.vector.tensor_sub(shift[:], bet_sb[:], t1[:])
            nc.vector.tensor_scalar(out_sb[:], out_sb[:], scale[:, 0:1],
                                    shift[:, 0:1], ALU.mult, ALU.add)
            nc.sync.dma_start(out_d[:], out_sb[:])
    nc.compile()
    return nc


def kernel(**inputs):
    x = np.ascontiguousarray(np.asarray(inputs["x"], dtype=np.float32))
    cp = np.asarray(inputs["control_points"], dtype=np.float32)
    conv_w = np.asarray(inputs["conv_w"], dtype=np.float32)
    gam = np.asarray(inputs["bn_gamma"], dtype=np.float32)
    bet = np.asarray(inputs["bn_beta"], dtype=np.float32)

    wts_f32 = _build_weights(cp, conv_w).reshape(KK * KK * N_TILES * 128, 128)
    if SPLIT_BF16:
        import ml_dtypes
        w_hi = wts_f32.astype(ml_dtypes.bfloat16)
        w_lo = (wts_f32 - w_hi.astype(np.float32)).astype(ml_dtypes.bfloat16)
        wts = np.ascontiguousarray(np.concatenate([w_hi, w_lo], axis=0))
    else:
        wts = np.ascontiguousarray(wts_f32)
    xpad = np.zeros((B, C, PW, PW), dtype=np.float32)
    xpad[:, :, 1:-1, 1:-1] = x
    xpad = xpad.reshape(B, C, PCOLS)

    if "nc" not in _cache:
        _cache["nc"] = _build_nc()
    nc = _cache["nc"]

    in_maps = [{"xpad": xpad[b], "wts": wts, "gam": gam.reshape(O, 1),
                "bet": bet.reshape(O, 1)} for b in range(B)]
    res = run_bass_kernel_spmd(nc, in_maps, list(range(N_CORES)))
    out = np.stack([res.results[b]["out"].reshape(O, HH, WW)
                    for b in range(B)], axis=0)
    return out.astype(np.float32)
